# revision 3
# baseline (speedup 1.0000x reference)
"""AttributeMemoryFusion kernel for 8x TRN2 NeuronCores (Bass/Tile), v6.

Per-sample attention over ragged memory + gated fusion:
    scores = mem @ h ; attn = softmax(mask(scores)) ; r = attn @ mem
    g = sigmoid(h @ Wg.T + r @ Ug.T + b) ; out = where(len>0, g*r+(1-g)*h, h)

v6 = v5 (bf16 mem/h) + length-sorted ragged packing:
  Samples are sorted by `lengths` on the host and dealt to cores so every
  core sees the same per-tile length cap profile (tile k holds samples from
  the k-th global length-octile; cap_k = that octile's max length). Only the
  first cap_k memory rows of each sample are packed and uploaded — rows
  m >= len are provably unused (masked in softmax, attn == 0 in r). With
  uniform lengths this cuts the dominant mem upload and the on-device
  HBM/dot/diag/matmul work to ~56% on top of v5's bf16 halving. The output
  permutation is undone on the host. Caps are computed from the actual
  inputs at run time (the Bass program is traced per cap profile).
"""

from contextlib import ExitStack

import numpy as np
import ml_dtypes

import concourse.bass as bass
import concourse.bacc as bacc
import concourse.mybir as mybir
import concourse.tile as tile
from concourse import masks
from concourse.bass_utils import run_bass_kernel_spmd

B, M, D = 8192, 64, 256
N_CORES = 8
BC = B // N_CORES      # samples per core
P = 128                # partitions / samples per tile
N_TILES = BC // P
BIG = 1.0e9
REPS = 1               # whole-batch repetitions (slope timing)

F32 = mybir.dt.float32
BF16 = mybir.dt.bfloat16
I32 = mybir.dt.int32
Alu = mybir.AluOpType
Act = mybir.ActivationFunctionType
AX = mybir.AxisListType


def _build_body(ctx, tc, io, caps):
    nc = tc.nc
    h_ap, mem_ap, len_ap, wg_ap, wgb_ap, ug_ap, ugb_ap, bg_ap, out_ap = io
    offs = np.concatenate([[0], np.cumsum([P * c for c in caps])])

    # ---- one-time constants ----
    const = ctx.enter_context(tc.tile_pool(name="const", bufs=1))
    ident = const.tile([P, P], F32)
    masks.make_identity(nc, ident[:])
    iota_m = const.tile([P, M], F32)
    nc.gpsimd.iota(
        iota_m[:], pattern=[[1, M]], base=0, channel_multiplier=0,
        allow_small_or_imprecise_dtypes=True,
    )
    ones_row = const.tile([1, P], BF16)
    nc.vector.memset(ones_row[:], 1.0)
    ident16 = const.tile([P, P], BF16)
    nc.vector.tensor_copy(ident16[:], ident[:])

    # ---- weights: load natural [o,i], transpose to lhsT layout [i_in, i_blk, o] ----
    wpool = ctx.enter_context(tc.tile_pool(name="weights", bufs=1))
    wg_nat = wpool.tile([P, 2, D], F32)
    ug_nat = wpool.tile([P, 2, D], F32)
    nc.sync.dma_start(wg_nat[:], wg_ap.rearrange("(a p) i -> p a i", p=P))
    nc.sync.dma_start(ug_nat[:], ug_ap.rearrange("(a p) i -> p a i", p=P))
    wgT = wpool.tile([P, 2, D], BF16)
    ugT = wpool.tile([P, 2, D], BF16)
    with tc.tile_pool(name="psw", bufs=2, space="PSUM") as psw:
        for nat, T in ((wg_nat, wgT), (ug_nat, ugT)):
            for ob in range(2):
                for ib in range(2):
                    pt = psw.tile([P, P], F32, tag="wtr")
                    nc.tensor.transpose(pt[:], nat[:, ob, ib * P:(ib + 1) * P], ident[:])
                    nc.scalar.copy(T[:, ib, ob * P:(ob + 1) * P], pt[:])

    # summed gate bias in transposed layout: [o_in, o_blk]
    bt0 = wpool.tile([P, 2], F32)
    bt1 = wpool.tile([P, 2], F32)
    bt2 = wpool.tile([P, 2], F32)
    bias_sb = wpool.tile([P, 2], F32)
    nc.sync.dma_start(bt0[:], wgb_ap.rearrange("(a p) -> p a", p=P))
    nc.sync.dma_start(bt1[:], ugb_ap.rearrange("(a p) -> p a", p=P))
    nc.sync.dma_start(bt2[:], bg_ap.rearrange("(a p) -> p a", p=P))
    nc.vector.tensor_add(bias_sb[:], bt0[:], bt1[:])
    nc.vector.tensor_add(bias_sb[:], bias_sb[:], bt2[:])
    nc.vector.tensor_scalar(bias_sb[:], bias_sb[:], 0.5, None, Alu.mult)

    # ---- pools ----
    mem_pool = ctx.enter_context(tc.tile_pool(name="mem", bufs=4))
    small = ctx.enter_context(tc.tile_pool(name="small", bufs=3))
    xstage = ctx.enter_context(tc.tile_pool(name="xstage", bufs=3))
    diag_pool = ctx.enter_context(tc.tile_pool(name="diag", bufs=16))
    out_pool = ctx.enter_context(tc.tile_pool(name="out", bufs=3))
    ps = ctx.enter_context(tc.tile_pool(name="ps", bufs=2, space="PSUM"))
    ps1 = ctx.enter_context(tc.tile_pool(name="ps1", bufs=1, space="PSUM"))

    # ---- whole-core upfront loads (tiny vs mem): h, lengths ----
    h_all = wpool.tile([P, N_TILES, D], BF16)
    nc.sync.dma_start(h_all[:], h_ap.rearrange("(t p) d -> p t d", p=P))
    lt_all = wpool.tile([P, N_TILES], I32)
    nc.sync.dma_start(lt_all[:], len_ap.rearrange("(t p) -> p t", p=P))
    lrow_all = wpool.tile([1, BC], I32)
    nc.sync.dma_start(lrow_all[:], len_ap.rearrange("(one b) -> one b", one=1))

    # prologue: per-tile +/-BIG softmax masks and empty-row gate masks
    ltf_all = wpool.tile([P, N_TILES], F32)
    nc.vector.tensor_copy(ltf_all[:], lt_all[:])
    maskbig_all = wpool.tile([P, N_TILES, M], F32)
    negrow_all = wpool.tile([1, BC], BF16)
    lrowf_all = wpool.tile([1, BC], F32)
    nc.vector.tensor_copy(lrowf_all[:], lrow_all[:])
    nc.vector.tensor_scalar(negrow_all[:], lrowf_all[:], 0.0, None, Alu.is_gt)
    nc.vector.tensor_scalar(negrow_all[:], negrow_all[:], BIG, BIG, Alu.mult, Alu.subtract)
    for t in range(N_TILES):
        mt_ = caps[t]
        nc.vector.tensor_scalar(
            maskbig_all[:, t, 0:mt_], iota_m[:, 0:mt_], ltf_all[:, t:t + 1],
            None, Alu.is_lt)
        nc.vector.tensor_scalar(
            maskbig_all[:, t, 0:mt_], maskbig_all[:, t, 0:mt_], 2.0 * BIG, BIG,
            Alu.mult, Alu.subtract)

    def scores_front(t):
        """DMA load (packed rows), scores, masked softmax, h-transpose."""
        b0 = t * P
        MT = caps[t]
        mck = mem_pool.tile([P, M, D], BF16, tag="mem")
        nc.sync.dma_start(
            mck[:, 0:MT, :],
            mem_ap[offs[t]:offs[t + 1], :].rearrange("(p m) d -> p m d", p=P),
        )

        ht = h_all[:, t, :]

        # ---- scores[b, m] = <mem[b, m, :], h[b, :]> (fused mult+accum) ----
        scratch = small.tile([P, D], BF16, tag="scratch")
        S = small.tile([P, M], F32, tag="S")
        for m in range(MT):
            nc.vector.scalar_tensor_tensor(
                out=scratch[:], in0=mck[:, m, :], scalar=1.0, in1=ht,
                op0=Alu.mult, op1=Alu.mult, accum_out=S[:, m:m + 1],
            )

        # ---- masked softmax over m: Sm = min(S, +/-BIG mask) ----
        Sm = small.tile([P, M], F32, tag="Sm")
        nc.vector.tensor_tensor(Sm[:, 0:MT], S[:, 0:MT], maskbig_all[:, t, 0:MT], Alu.min)
        negmax = small.tile([P, 1], F32, tag="negmax")
        nc.vector.tensor_reduce(negmax[:], Sm[:, 0:MT], AX.X, Alu.max, negate=True)
        E = small.tile([P, M], F32, tag="E")
        nc.scalar.activation(E[:, 0:MT], Sm[:, 0:MT], Act.Exp, bias=negmax[:], scale=1.0)
        ssum = small.tile([P, 1], F32, tag="ssum")
        nc.vector.tensor_reduce(ssum[:], E[:, 0:MT], AX.X, Alu.add)
        rinv = small.tile([P, 1], F32, tag="rinv")
        nc.vector.reciprocal(rinv[:], ssum[:])
        attn = xstage.tile([P, M], F32, tag="attn")
        nc.vector.tensor_scalar(attn[:, 0:MT], E[:, 0:MT], rinv[:], None, Alu.mult)

        # h transpose (only needs ht)
        pt_h = ps1.tile([P, 2, P], BF16, tag="pth")
        hT = xstage.tile([P, 2, P], BF16, tag="hT")
        for k in range(2):
            nc.tensor.transpose(pt_h[:, k, :], ht[:, k * P:(k + 1) * P], ident16[:])
            nc.scalar.copy(hT[:, k, :], pt_h[:, k, :])

        return dict(ht=ht, hT=hT, negrow=negrow_all[:, b0:b0 + P],
                    attn=attn, mck=mck, b0=b0, MT=MT,
                    last=(t == N_TILES - 1))

    def r_front(st):
        """r[b, :] = sum_m attn[b, m] * mem[b, m, :], on TensorE via
        diag(attn_m) bf16 matmuls accumulated in PSUM."""
        attn, mck, MT, last = st["attn"], st["mck"], st["MT"], st["last"]
        R_ps = ps.tile([P, D], F32, tag="Rps")
        for m in range(MT):
            dg = diag_pool.tile([P, P], BF16, tag="dg")
            if last and m % 2 == 1:
                nc.vector.tensor_scalar(dg[:], ident[:], attn[:, m:m + 1], None, Alu.mult)
            else:
                nc.scalar.activation(dg[:], ident[:], Act.Copy, bias=0.0,
                                     scale=attn[:, m:m + 1])
            nc.tensor.matmul(
                R_ps[:], dg[:], mck[:, m, :],
                start=(m == 0), stop=(m == MT - 1),
            )
        st["R_ps"] = R_ps
        return st

    def backend(st):
        """Combine r, gate matmuls, sigmoid, blend, store."""
        ht, R_ps, hT, negrow, b0 = (
            st["ht"], st["R_ps"], st["hT"], st["negrow"], st["b0"]
        )
        R = small.tile([P, D], F32, tag="R")
        nc.scalar.copy(R[:], R_ps[:])
        Rb = small.tile([P, D], BF16, tag="Rb")
        nc.vector.tensor_copy(Rb[:], R[:])

        pt_r = ps1.tile([P, 2, P], BF16, tag="ptr")
        rT = small.tile([P, 2, P], BF16, tag="rT")
        for k in range(2):
            nc.tensor.transpose(pt_r[:, k, :], Rb[:, k * P:(k + 1) * P], ident16[:])
            nc.scalar.copy(rT[:, k, :], pt_r[:, k, :])

        # ---- gate preactivation in PSUM: Wg@hT + Ug@rT + ones x negrow ----
        G = ps.tile([P, 2, P], F32, tag="G")
        for ob in range(2):
            for ib in range(2):
                nc.tensor.matmul(
                    G[:, ob, :],
                    wgT[:, ib, ob * P:(ob + 1) * P],
                    hT[:, ib, :],
                    start=(ib == 0), stop=False,
                )
            for ib in range(2):
                nc.tensor.matmul(
                    G[:, ob, :],
                    ugT[:, ib, ob * P:(ob + 1) * P],
                    rT[:, ib, :],
                    start=False, stop=False,
                )
            nc.tensor.matmul(G[:, ob, :], ones_row[:], negrow[:],
                             start=False, stop=True)

        # y = tanh((pre + bias)/2); g = 0.5*(1+y) folded into the blend.
        gT = small.tile([P, 2, P], F32, tag="gT")
        for ob in range(2):
            nc.scalar.activation(
                gT[:, ob, :], G[:, ob, :], Act.Tanh,
                bias=bias_sb[:, ob:ob + 1], scale=0.5,
            )

        GB = ps.tile([P, 2, P], F32, tag="GB")
        for ob in range(2):
            nc.tensor.transpose(GB[:, ob, :], gT[:, ob, :], ident[:])

        # ---- out = h + 0.5*(1+y)*(r-h) ----
        T1 = small.tile([P, D], F32, tag="T1")
        nc.vector.tensor_tensor(T1[:], R[:], ht, Alu.subtract)
        T2 = small.tile([P, D], F32, tag="T2")
        nc.vector.scalar_tensor_tensor(
            out=T2[:], in0=GB[:].rearrange("p a b -> p (a b)"), scalar=1.0,
            in1=T1[:], op0=Alu.add, op1=Alu.mult,
        )
        ot = out_pool.tile([P, D], F32, tag="ot")
        nc.vector.scalar_tensor_tensor(
            out=ot[:], in0=T2[:], scalar=0.5, in1=ht, op0=Alu.mult, op1=Alu.add,
        )
        nc.sync.dma_start(out_ap[b0:b0 + P, :], ot[:])

    # 3-stage software pipeline.
    for _rep in range(REPS):
        stages = []
        for t in range(N_TILES):
            stages.append(scores_front(t))
            if t >= 1:
                r_front(stages[t - 1])
            if t >= 2:
                backend(stages[t - 2])
        r_front(stages[N_TILES - 1])
        backend(stages[N_TILES - 2])
        backend(stages[N_TILES - 1])


_CACHE = {}


def _get_nc(caps):
    key = ("nc", REPS, caps)
    if key in _CACHE:
        return _CACHE[key]
    total_rows = int(P * sum(caps))
    nc = bacc.Bacc("TRN2", target_bir_lowering=False, debug=False, num_devices=N_CORES)
    h_ap = nc.dram_tensor("h_tilde", [BC, D], BF16, kind="ExternalInput").ap()
    mem_ap = nc.dram_tensor("mem", [total_rows, D], BF16, kind="ExternalInput").ap()
    len_ap = nc.dram_tensor("lengths", [BC], I32, kind="ExternalInput").ap()
    wg_ap = nc.dram_tensor("Wg_w", [D, D], F32, kind="ExternalInput").ap()
    wgb_ap = nc.dram_tensor("Wg_b", [D], F32, kind="ExternalInput").ap()
    ug_ap = nc.dram_tensor("Ug_w", [D, D], F32, kind="ExternalInput").ap()
    ugb_ap = nc.dram_tensor("Ug_b", [D], F32, kind="ExternalInput").ap()
    bg_ap = nc.dram_tensor("b_g", [D], F32, kind="ExternalInput").ap()
    out_ap = nc.dram_tensor("out", [BC, D], F32, kind="ExternalOutput").ap()
    io = (h_ap, mem_ap, len_ap, wg_ap, wgb_ap, ug_ap, ugb_ap, bg_ap, out_ap)
    with tile.TileContext(nc) as tc:
        with ExitStack() as ctx:
            _build_body(ctx, tc, io, caps)
    nc.finalize()
    _CACHE[key] = nc
    return nc


def _plan(lengths):
    """Sort samples by length; deal global octile blocks across cores so
    every core has the same per-tile cap profile. Returns (perm[B] of
    sample ids in device order core-major, caps[N_TILES])."""
    order = np.argsort(lengths, kind="stable")
    caps = []
    perm = np.empty(B, dtype=np.int64)
    for k in range(N_TILES):
        blk = order[k * (P * N_CORES):(k + 1) * (P * N_CORES)]
        caps.append(int(max(1, lengths[blk].max())))
        # core c, tile k, partition p <- blk[p * N_CORES + c]
        for c in range(N_CORES):
            perm[c * BC + k * P: c * BC + (k + 1) * P] = blk[c::N_CORES]
    return perm, tuple(caps)


def _make_in_maps(inputs):
    lengths_full = np.asarray(inputs["lengths"], dtype=np.int32)
    perm, caps = _plan(lengths_full)
    h = np.asarray(inputs["h_tilde"], dtype=np.float32).astype(ml_dtypes.bfloat16)
    mem = np.asarray(inputs["mem"], dtype=np.float32).astype(ml_dtypes.bfloat16)
    shared = {
        "Wg_w": np.ascontiguousarray(np.asarray(inputs["Wg_w"], dtype=np.float32)),
        "Wg_b": np.ascontiguousarray(np.asarray(inputs["Wg_b"], dtype=np.float32)),
        "Ug_w": np.ascontiguousarray(np.asarray(inputs["Ug_w"], dtype=np.float32)),
        "Ug_b": np.ascontiguousarray(np.asarray(inputs["Ug_b"], dtype=np.float32)),
        "b_g": np.ascontiguousarray(np.asarray(inputs["b_g"], dtype=np.float32)),
    }
    in_maps = []
    for c in range(N_CORES):
        ids = perm[c * BC:(c + 1) * BC]
        packed = np.concatenate([
            mem[ids[k * P:(k + 1) * P], :caps[k], :].reshape(P * caps[k], D)
            for k in range(N_TILES)
        ], axis=0)
        in_maps.append({
            "h_tilde": np.ascontiguousarray(h[ids]),
            "mem": np.ascontiguousarray(packed),
            "lengths": np.ascontiguousarray(lengths_full[ids]),
            **shared,
        })
    return in_maps, perm, caps


def run(inputs, **kwargs):
    in_maps, perm, caps = _make_in_maps(inputs)
    nc = _get_nc(caps)
    res = run_bass_kernel_spmd(nc, in_maps, list(range(N_CORES)), **kwargs)
    return res, perm


def kernel(**inputs) -> np.ndarray:
    res, perm = run(inputs)
    permuted = np.concatenate(
        [res.results[c]["out"] for c in range(N_CORES)], axis=0)
    out = np.empty_like(permuted)
    out[perm] = permuted
    return out
